# revision 45
# baseline (speedup 1.0000x reference)
"""Gated GQA self-attention with KV cache, tensor-parallel over heads on 8
Trainium2 NeuronCores.

Reference computation (fp32):
    q = rms_norm((x @ w_q.T).reshape(B,L,H,HD))      # per-head rms over HD
    k = rms_norm((x @ w_k.T).reshape(B,L,HKV,HD))
    v = (x @ w_v.T).reshape(B,L,HKV,HD)
    k_t/v_t = concat(cache, new) over seq -> [B,HKV,S,HD]
    o = softmax(q @ k_t.T / sqrt(HD)) @ v_t          # full (non-causal)
    o *= sigmoid(x[..., :16] @ w_gate.T)             # per-head gate
    y = o.reshape(B,L,D) @ w_out.T

Sharding: core c owns q heads {2c, 2c+1} and kv group g=c//2.  Each core
computes its heads' attention plus the partial out-projection
y_c = o_c @ w_out[:, cols_c].T; the host sums the 8 partials.

v3 design notes:
  * All matmul operands bf16 (fp32 PSUM accumulation); measured MM issue
    rate is N/2.4+2.5 with LDWEIGHTS fully hidden as long as every
    matmul keeps M=128 -- so all reduction-by-ones matmuls use [128,128]
    ones stationaries.  The replicated output rows double as the
    partition-broadcast of the result, so rms/softmax scales never need
    rank-1 matmuls, DRAM transposes, or 1-lane row ops.
  * rms scale: 1/sqrt(colsum(q^2)) via ACT Sqrt (free affine folds the
    1/HD for k) + DVE reciprocal_approx_fast on the full [128,512] tile.
    The softmax denominator reciprocal is the same DVE op in PSUM.
    The eps bias (1e-6 vs ssq~128) is numerically irrelevant; dropped.
  * Attention interleaves heads h0/h1 of one (b, lc2): per s-chunk-pair
    the PE stream is [scores h0][av+den h1][scores h1][av+den h0], so
    the 1147ns 1024-wide exp gets ~2.6us of slack with only two
    [128,1024] score buffers.
  * Gates ride the attention-output evacuation (TT mul instead of copy);
    the out-projection accumulates h0+h1 into one PSUM tile per y block;
    y is staged in [128,2048] tiles so only 16 output DMAs are issued.
  * PSUM: scores 2x[128,1024] + acc 2x[128,512] + misc 2x[128,512] =
    exactly 8 banks.
"""

from contextlib import ExitStack

import numpy as np

import concourse.bass as bass
import concourse.tile as tile
from concourse import bacc, mybir
from concourse.bass_utils import run_bass_kernel_spmd

BF16 = mybir.dt.bfloat16
F32R = mybir.dt.float32r
F32 = mybir.dt.float32
AF = mybir.ActivationFunctionType
ALU = mybir.AluOpType

B, L, D = 2, 1024, 2048
H, HKV, HD = 16, 4, 128
CACHE = 1024
BL = B * L                  # 2048
S = CACHE + L               # 2048
NCORES = 8
QH = H // NCORES            # 2 q heads per core
JC = QH * HD                # 256 out-proj contraction cols per core

ND = D // 128               # 16 contraction chunks for the projections
NLP = 4                     # l-chunks of 512 for the x stream
NSC = CACHE // 128          # 8 cached s chunks per batch

_CACHED_NC = None


def _build_core_program():
    """One SPMD program; per-core differences are input data only."""
    nc = bacc.Bacc("TRN2", target_bir_lowering=False, debug=False)

    xt = nc.dram_tensor("xt", [D, BL], BF16, kind="ExternalInput").ap()
    wqkv = nc.dram_tensor("wqkv", [D, 4 * HD], BF16, kind="ExternalInput").ap()
    wo = nc.dram_tensor("wo", [JC, D], BF16, kind="ExternalInput").ap()
    # per-head gate weight columns replicated 128x so the gate matmul has
    # M=128 and its output is already partition-broadcast
    wgr = nc.dram_tensor("wgr", [16, QH * 128], BF16, kind="ExternalInput").ap()
    ckt = nc.dram_tensor("ckt", [B, HD, CACHE], BF16, kind="ExternalInput").ap()
    cv = nc.dram_tensor("cv", [B, CACHE, HD], BF16, kind="ExternalInput").ap()
    # [:, 0:128] identity (PE transposes), [:, 128:256] all-ones (sums)
    cb_in = nc.dram_tensor("cb", [128, 256], BF16, kind="ExternalInput").ap()
    onesf_in = nc.dram_tensor("onesf", [128, 128], F32R, kind="ExternalInput").ap()
    y = nc.dram_tensor("y", [BL, D], BF16, kind="ExternalOutput").ap()

    with tile.TileContext(nc) as tc, ExitStack() as ctx:
        singles = ctx.enter_context(tc.tile_pool(name="singles", bufs=1))
        xtp = ctx.enter_context(tc.tile_pool(name="xtp", bufs=2))
        work = ctx.enter_context(tc.tile_pool(name="work", bufs=2))
        exp_pool = ctx.enter_context(tc.tile_pool(name="exp", bufs=4))
        cachep = ctx.enter_context(tc.tile_pool(name="cachep", bufs=1))

        psS = ctx.enter_context(tc.tile_pool(name="psS", bufs=2, space="PSUM"))
        psAcc = ctx.enter_context(tc.tile_pool(name="psAcc", bufs=2, space="PSUM"))
        psM = ctx.enter_context(tc.tile_pool(name="psM", bufs=2, space="PSUM"))

        lowp = nc.allow_low_precision(reason="bf16 operands are intended")
        ctx.enter_context(lowp)

        cb = singles.tile([128, 256], BF16)
        nc.sync.dma_start(out=cb, in_=cb_in)
        identb = cb[:, 0:128]
        ones128 = cb[:, 128:256]
        onesf = singles.tile([128, 128], F32R)
        nc.sync.dma_start(out=onesf, in_=onesf_in)

        # HAM warmup: junk matmuls during the DMA lead-in push the PE
        # activity monitor past its ~3.4us window so the real projection
        # stream starts at 2.4GHz instead of half clock.  The read-back
        # releases the PSUM slot (a write-only tile would pin it).
        warm = psM.tile([128, 256], F32, tag="psM", name="warm")
        for _ in range(12):
            nc.tensor.matmul(warm, cb[:, 0:128], cb, start=True, stop=True)
        warm_sb = work.tile([128, 256], F32, tag="rt", name="warm_sb")
        nc.vector.tensor_copy(warm_sb, warm)

        wgr_sb = singles.tile([16, QH * 128], BF16)
        nc.scalar.dma_start(out=wgr_sb, in_=wgr[0:16, :])
        wqkv_sb = singles.tile([128, ND, 4 * HD], BF16)
        wqkv_r = wqkv.rearrange("(k p) j -> p k j", p=128)
        wo_sb = singles.tile([128, QH, D], BF16)

        # persistent activations, feature-on-partition
        qkvt = singles.tile([128, 4, BL], BF16)       # jc: q0, q1, k, v
        otg = singles.tile([128, B, QH, 2, 512], BF16)  # gated attention out
        # sigmoid gates, partition-broadcast: chunk c = b*2 + lc2
        gates_sb = singles.tile([128, QH, 4, 512], F32)
        xg = singles.tile([16, BL], BF16)              # x[..., :16] for gates
        cache_tiles = {}

        def emit_prefetch():
            """Non-critical loads on the sync HWDGE (the scalar queue is
            busy streaming wqkv + odd x tiles)."""
            nc.sync.dma_start(out=xg, in_=xt[0:16, :])
            nc.sync.dma_start(
                out=wo_sb, in_=wo.rearrange("(h p) m -> p h m", p=128)
            )
            for b in range(B):
                ck_sb = cachep.tile(
                    [128, CACHE], BF16, tag=f"ck{b}", name=f"ck{b}"
                )
                nc.sync.dma_start(out=ck_sb, in_=ckt[b])
                cv_sb = cachep.tile(
                    [128, NSC, HD], BF16, tag=f"cv{b}", name=f"cv{b}"
                )
                nc.sync.dma_start(
                    out=cv_sb, in_=cv[b].rearrange("(i p) d -> p i d", p=128)
                )
                vnew = cachep.tile(
                    [128, NSC, HD], BF16, tag=f"vn{b}", name=f"vn{b}"
                )
                cache_tiles[b] = (ck_sb, cv_sb, vnew)

        # ---- phase 1: projections -------------------------------------
        def vnew_transposes(b):
            _, _, vnew = cache_tiles[b]
            for i in range(NSC):
                tp = psM.tile([128, 128], BF16, tag="psM", name=f"tp{b}{i}")
                nc.tensor.transpose(
                    tp, qkvt[:, 3, b * L + i * 128 : b * L + i * 128 + 128],
                    identb,
                )
                nc.vector.tensor_copy(vnew[:, i, :], tp)

        def gates():
            # sigmoid as 1/(1+exp(-x)): the ACT exp stays in the one table
            # set the whole kernel uses; the reciprocal rides DVE
            for h in range(QH):
                for c in range(4):
                    gsl = slice(c * 512, c * 512 + 512)
                    gp = psM.tile([128, 512], F32, tag="psM", name=f"gp{h}{c}")
                    nc.tensor.matmul(
                        gp,
                        wgr_sb[:, h * 128 : h * 128 + 128],
                        xg[:, gsl],
                        start=True,
                        stop=True,
                    )
                    ge = work.tile([128, 512], F32, tag="rt", name=f"ge{h}{c}")
                    nc.scalar.activation(ge, gp, AF.Exp, scale=-1.0)
                    nc.vector.tensor_scalar_add(ge, ge, 1.0)
                    nc.vector.reciprocal_approx_fast(
                        out=gates_sb[:, h, c, :], in_=ge
                    )

        xt_r = xt.rearrange("(k p) l -> p k l", p=128)

        def load_xtile(lc):
            # alternate HWDGE queues so the x stream and the prefetches
            # never serialize behind each other
            eng = nc.sync if lc % 2 == 0 else nc.scalar
            sl = slice(lc * 512, lc * 512 + 512)
            xtile = xtp.tile([128, ND, 512], BF16, tag="xt", name=f"xt{lc}")
            for kq in range(4):
                eng.dma_start(
                    out=xtile[:, kq * 4 : kq * 4 + 4, :],
                    in_=xt_r[:, kq * 4 : kq * 4 + 4, sl],
                )
            return xtile

        for kq in range(4):
            nc.scalar.dma_start(
                out=wqkv_sb[:, kq * 4 : kq * 4 + 4, :],
                in_=wqkv_r[:, kq * 4 : kq * 4 + 4, :],
            )
        xtiles = [load_xtile(0)]
        emit_prefetch()
        xtiles.append(load_xtile(1))

        def proj_chunk(lc, pending=None):
            sl = slice(lc * 512, lc * 512 + 512)
            xtile = xtiles[lc]
            # First chunk of a batch: v first (unblocks attention prep).
            # Second chunk: v LAST, so the rms Sqrts finish ~6us of PE
            # work before the batch's first exp can be ready -- otherwise
            # the scheduler interleaves them and the Sqrt/Exp ACT table
            # sets ping-pong at 1283ns per swap.
            for jc in ((3, 2, 1, 0) if lc % 2 == 0 else (2, 1, 0, 3)):
                pp = psAcc.tile([128, 512], F32, tag="acc", name=f"pp{lc}_{jc}")
                for kk in range(ND):
                    nc.tensor.matmul(
                        pp,
                        wqkv_sb[:, kk, jc * 128 : jc * 128 + 128],
                        xtile[:, kk, :],
                        start=(kk == 0),
                        stop=(kk == ND - 1),
                    )
                nc.vector.tensor_copy(qkvt[:, jc, sl], pp)
                if pending is not None:
                    pending()        # previous attention pair's h1 finish
                    pending = None
                if jc != 3:  # q0, q1, k: rms normalize over HD
                    qsl_ = qkvt[:, jc, sl]
                    sq = work.tile([128, 512], BF16, tag="sq", name=f"sq{lc}_{jc}")
                    # q factor 1/sqrt(ssq) = rms * 1/sqrt(HD) (score scale
                    # folded); k factor 1/sqrt(ssq/HD) via the Ln scale
                    nc.vector.tensor_mul(sq, qsl_, qsl_)
                    ssq = psM.tile([128, 512], F32, tag="psM", name="ssq")
                    nc.tensor.matmul(ssq, ones128, sq, start=True, stop=True)
                    rt = work.tile([128, 512], F32, tag="rt", name=f"rt{lc}{jc}")
                    nc.scalar.activation(
                        rt, ssq, AF.Sqrt,
                        scale=(1.0 if jc < QH else 1.0 / HD),
                    )
                    bc = work.tile([128, 512], F32, tag="bc", name=f"bc{lc}{jc}")
                    nc.vector.reciprocal_approx_fast(out=bc, in_=rt)
                    nc.vector.tensor_mul(qsl_, qsl_, bc)

        # batch-interleaved schedule: proj(b0) -> attention(b0) ->
        # proj(b1) -> attention(b1), with outproj(b0) between the two b1
        # attention pairs.  b1's Sqrt work is never data-ready while b0's
        # exps run, so the ACT table sets stay put; attention starts
        # ~35us earlier; y(b0) DMAs stream under b1's attention.
        proj_chunk(0)
        proj_chunk(1)
        vnew_transposes(0)
        gates()

        # ---- phase 2: attention ---------------------------------------
        # per (b, lc2): heads h0/h1 interleaved at s-chunk-pair granularity
        def attn_pair(b, lc2, pending, inject):
            boff = b * L
            off = boff + lc2 * 512
            ck_sb, cv_sb, vnew = cache_tiles[b]
            qsl = [qkvt[:, h, off : off + 512] for h in range(QH)]
            ot = [
                psAcc.tile([128, 512], F32, tag="acc", name=f"ot{b}{lc2}{h}")
                for h in range(QH)
            ]
            den0 = psM.tile([128, 512], F32, tag="psM", name=f"dn{b}{lc2}")
            # h1's denominator accumulates on DVE (f32) to keep the PE
            # stream at 10 matmuls per s-chunk-pair -- the 2x1147ns exp
            # then sets the cycle, not PE
            dacc = work.tile([128, 512], F32R, tag="dacc", name=f"da{b}{lc2}")

            def av_den(h, scp, ex):
                for half in range(2):
                    sc = 2 * scp + half
                    vx = (
                        cv_sb[:, sc, :] if sc < NSC else vnew[:, sc - NSC, :]
                    )
                    nc.tensor.matmul(
                        ot[h], vx, ex[:, half * 512 : half * 512 + 512],
                        start=(scp == 0 and half == 0),
                        stop=(scp == 7 and half == 1),
                    )
                if h == 0:
                    for half in range(2):
                        nc.tensor.matmul(
                            den0, ones128,
                            ex[:, half * 512 : half * 512 + 512],
                            start=(scp == 0 and half == 0),
                            stop=(scp == 7 and half == 1),
                        )
                else:
                    for half in range(2):
                        exh = ex[:, half * 512 : half * 512 + 512]
                        if scp == 0 and half == 0:
                            nc.vector.tensor_copy(dacc, exh)
                        else:
                            nc.vector.tensor_add(dacc, dacc, exh)

            def scores(h, scp):
                sp = psS.tile([128, 1024], F32, tag="psS", name=f"sp{h}{scp}")
                for half in range(2):
                    sc = 2 * scp + half
                    if sc < NSC:
                        kT = ck_sb[:, sc * 128 : sc * 128 + 128]
                    else:
                        j = boff + (sc - NSC) * 128
                        kT = qkvt[:, 2, j : j + 128]
                    nc.tensor.matmul(
                        sp[:, half * 512 : half * 512 + 512],
                        kT, qsl[h], start=True, stop=True,
                    )
                ex = exp_pool.tile(
                    [128, 1024], BF16, tag="ex", name=f"ex{h}{scp}"
                )
                nc.scalar.activation(ex, sp, AF.Exp)
                return ex

            pend = [None, None]
            for scp in range(8):
                for h in range(QH):
                    o = 1 - h
                    exh = scores(h, scp)
                    if pending is not None:
                        pending()          # previous pair's h1 finish
                        pending = None
                    if pend[o] is not None:
                        av_den(o, *pend[o])
                        pend[o] = None
                    pend[h] = (scp, exh)
                    if h == 1 and inject is not None:
                        next(inject, None)  # one out-proj group per scp
            for h in (0, 1):
                if pend[h] is not None:
                    av_den(h, *pend[h])
            # evacuate h0 at the pair end: gate rides the copy, then 1/den
            # (the den tile is partition-broadcast, no reshape needed)
            nc.vector.tensor_mul(
                otg[:, b, 0, lc2, :], ot[0], gates_sb[:, 0, b * 2 + lc2, :]
            )
            nc.vector.reciprocal_approx_fast(out=den0, in_=den0)
            nc.vector.tensor_mul(
                otg[:, b, 0, lc2, :], otg[:, b, 0, lc2, :], den0
            )

            def finish1():
                # h1: sum matmul over the DVE-accumulated denominator,
                # then the same gated evacuation; deferred into the next
                # PE block so the pair boundary never stalls
                den1 = psM.tile([128, 512], F32, tag="psM", name=f"d1{b}{lc2}")
                nc.tensor.matmul(den1, onesf, dacc, start=True, stop=True)
                nc.vector.tensor_mul(
                    otg[:, b, 1, lc2, :], ot[1],
                    gates_sb[:, 1, b * 2 + lc2, :],
                )
                nc.vector.reciprocal_approx_fast(out=den1, in_=den1)
                nc.vector.tensor_mul(
                    otg[:, b, 1, lc2, :], otg[:, b, 1, lc2, :], den1
                )

            return finish1

        # ---- phase 3: partial out-projection --------------------------
        def outproj_half(b, lc2, pending=None):
            # rotate yp tiles through all three PSUM pools (scores + misc
            # are idle here): 5 slots in flight hide the evacuation
            # round-trip that starved a 2-slot rotation
            yp_pools = [
                (psAcc, "acc"), (psS, "psS"), (psM, "psM"), (psS, "psS"),
            ]
            if pending is not None:
                # the attention pair's h1 finish must precede any yp that
                # reads its otg block, and its DVE evac must precede the
                # yp copies on the DVE queue (slot-release ordering)
                pending()
            for li in range(4):
                row0 = b * L + lc2 * 512 + li * 128
                ysb = work.tile(
                    [128, 2048], BF16, tag="ysb", name="ysb", bufs=3
                )
                for mc in range(4):
                    pool, tag = yp_pools[mc]
                    yp = pool.tile(
                        [128, 512], F32, tag=tag, name=f"yp{mc}"
                    )
                    for h in range(QH):
                        nc.tensor.matmul(
                            yp,
                            otg[:, b, h, lc2, li * 128 : li * 128 + 128],
                            wo_sb[:, h, mc * 512 : mc * 512 + 512],
                            start=(h == 0),
                            stop=(h == 1),
                        )
                    ysl = ysb[:, mc * 512 : mc * 512 + 512]
                    if mc % 2:
                        nc.scalar.copy(ysl, yp)
                    else:
                        nc.vector.tensor_copy(ysl, yp)
                nc.sync.dma_start(
                    out=y[row0 : row0 + 128, :], in_=ysb
                )

        f = attn_pair(0, 0, None, None)
        f = attn_pair(0, 1, f, None)
        # b1's x tiles stream in under b0's attention
        xtiles.append(load_xtile(2))
        xtiles.append(load_xtile(3))
        proj_chunk(2, pending=f)
        proj_chunk(3)
        vnew_transposes(1)
        # only b1/lc2=1's y block (2.1MB) remains after the last pair, so
        # the final y drain is short
        f = attn_pair(1, 0, None, None)
        outproj_half(0, 0, pending=f)
        outproj_half(0, 1)
        outproj_half(1, 0)
        f = attn_pair(1, 1, None, None)
        outproj_half(1, 1, pending=f)

    nc.compile()
    return nc


def _get_nc():
    global _CACHED_NC
    if _CACHED_NC is None:
        _CACHED_NC = _build_core_program()
    return _CACHED_NC


def make_in_maps(x, w_q, w_k, w_v, w_out, w_gate, cache_k, cache_v):
    import ml_dtypes

    bf16 = ml_dtypes.bfloat16
    xt = np.ascontiguousarray(x.reshape(BL, D).T.astype(bf16))
    cb = np.concatenate(
        [np.eye(128, dtype=np.float32), np.ones((128, 128), np.float32)],
        axis=1,
    ).astype(bf16)
    in_maps = []
    for c in range(NCORES):
        g = c // 2
        wq_c = w_q[c * JC : (c + 1) * JC]                      # [256, D]
        wk_c = w_k[g * HD : (g + 1) * HD]                      # [128, D]
        wv_c = w_v[g * HD : (g + 1) * HD]
        wqkv_c = np.ascontiguousarray(
            np.concatenate([wq_c, wk_c, wv_c], axis=0).T.astype(bf16)
        )
        wo_c = np.ascontiguousarray(
            w_out[:, c * JC : (c + 1) * JC].T.astype(bf16)
        )
        wg_c = w_gate[c * QH : (c + 1) * QH].T.astype(np.float32)  # [16, 2]
        wgr_c = np.ascontiguousarray(
            np.repeat(wg_c, 128, axis=1).astype(bf16)          # [16, 256]
        )
        ckt_c = np.ascontiguousarray(
            cache_k[:, g].transpose(0, 2, 1).astype(bf16)
        )
        cv_c = np.ascontiguousarray(cache_v[:, g].astype(bf16))
        in_maps.append(
            {
                "xt": xt,
                "wqkv": wqkv_c,
                "wo": wo_c,
                "wgr": wgr_c,
                "ckt": ckt_c,
                "cv": cv_c,
                "cb": cb,
                "onesf": np.ones((128, 128), np.float32),
            }
        )
    return in_maps


def kernel(x, w_q, w_k, w_v, w_out, w_gate, cache_k, cache_v, _run_kwargs=None):
    in_maps = make_in_maps(x, w_q, w_k, w_v, w_out, w_gate, cache_k, cache_v)
    nc = _get_nc()
    res = run_bass_kernel_spmd(
        nc, in_maps, core_ids=list(range(NCORES)), **(_run_kwargs or {})
    )
    acc = np.zeros((BL, D), dtype=np.float32)
    for c in range(NCORES):
        acc += res.results[c]["y"].astype(np.float32)
    out = acc.reshape(B, L, D)
    if _run_kwargs:
        kernel.last_results = res
    return out
